# revision 3
# baseline (speedup 1.0000x reference)
"""Trainium2 kernel for nn_LinearMem: bit-sliced int8-quantized linear layer.

Math: the reference splits round(x/sx) and round(w.T/sw) into two's-complement
bit-planes (widths 1,1,2,4) and recombines 16 per-slice-pair matmuls with
2^shift weights.  That recombination is exactly sum_i 2^sh_i * plane_i == q,
so the whole einsum equals qx @ qw^T with qx = round(x/sx), qw = round(w/sw)
(clip to +-127 is a no-op since |x|/sx <= 127 by construction).  Every product
and partial sum is an integer < 2^24, so a bf16 x bf16 matmul with f32 PSUM
accumulation reproduces the reference bitwise (int8 values are exact in bf16).

Quantization itself needs an exact IEEE f32 divide to match the reference's
rounding; Trainium has no divide instruction on any engine (DVE/ACT/GPSIMD ISA
all reject AluOpType.divide), so the int8 quantization + shard layout prep is
done host-side (as in real quantized inference, where weights are quantized
offline).  The device does all 17 GFLOP of matmul plus dequantize + bias.

Distribution (8 NeuronCores, tensor-parallel 2x4 grid):
  core c = (i, j): i = c//4 selects token rows (M/2 = 1024), j = c%4 selects
  out_features (N/4 = 512).  Each core receives its pre-transposed [K, M_c]
  activation slice and [K, N_c] weight slice in bf16, accumulates
  out[m, n] = sum_k qxT[k, m] * qwT[k, n] over 16 K-blocks into 8 persistent
  PSUM banks, then dequantizes (ACT scale) + adds bias (DVE) and writes its
  [1024, 512] f32 output block.  Host reassembles the 2x4 grid.
"""

import sys

if "/opt/trn_rl_repo" not in sys.path:
    sys.path.insert(0, "/opt/trn_rl_repo")

import ml_dtypes
import numpy as np

import concourse.bacc as bacc
import concourse.mybir as mybir
import concourse.tile as tile
from concourse.bass_utils import run_bass_kernel_spmd

M, K, N = 2048, 2048, 2048
PM, PN = 2, 4  # grid: M split PM ways, N split PN ways
MS, NS = M // PM, N // PN  # per-core shard sizes: 1024, 512

F32 = mybir.dt.float32
BF16 = mybir.dt.bfloat16


def _build_program():
    nc = bacc.Bacc("TRN2", target_bir_lowering=False, debug=False, num_devices=8)

    qx_in = nc.dram_tensor("qxt_sh", [K, MS], BF16, kind="ExternalInput")
    qw_in = nc.dram_tensor("qwt_sh", [K, NS], BF16, kind="ExternalInput")
    b_in = nc.dram_tensor("b_sh", [1, NS], F32, kind="ExternalInput")
    scl_in = nc.dram_tensor("scl", [1, 4], F32, kind="ExternalInput")
    out_t = nc.dram_tensor("out_sh", [MS, NS], F32, kind="ExternalOutput")

    MT = MS // 128  # 8 m-tiles
    KT = K // 128  # 16 k-blocks

    with tile.TileContext(nc) as tc:
        with (
            tc.tile_pool(name="const", bufs=1) as const,
            tc.tile_pool(name="mm", bufs=3) as mm,
            tc.tile_pool(name="out", bufs=3) as op,
            tc.tile_pool(name="psum", bufs=1, space="PSUM") as ps,
        ):
            scl_row = const.tile([1, 4], F32, tag="scl_row")
            nc.sync.dma_start(scl_row[:], scl_in[:])
            sclb = const.tile([128, 4], F32, tag="sclb")
            nc.gpsimd.partition_broadcast(sclb[:], scl_row[:], channels=128)
            s_ap = sclb[:, 0:1]  # dequant scale sx*sw

            bias_row = const.tile([1, NS], F32, tag="bias_row")
            nc.sync.dma_start(bias_row[:], b_in[:])
            bias_b = const.tile([128, NS], F32, tag="bias_b")
            nc.gpsimd.partition_broadcast(bias_b[:], bias_row[:], channels=128)

            acc = [
                ps.tile([128, NS], F32, tag=f"acc{m}", name=f"acc{m}")
                for m in range(MT)
            ]
            for kb in range(KT):
                xt = mm.tile([128, MS], BF16, tag="xt")
                nc.sync.dma_start(xt[:], qx_in[kb * 128 : (kb + 1) * 128, :])
                wt = mm.tile([128, NS], BF16, tag="wt")
                nc.sync.dma_start(wt[:], qw_in[kb * 128 : (kb + 1) * 128, :])
                for mb in range(MT):
                    nc.tensor.matmul(
                        acc[mb][:],
                        xt[:, mb * 128 : (mb + 1) * 128],
                        wt[:],
                        start=(kb == 0),
                        stop=(kb == KT - 1),
                    )

            for mb in range(MT):
                o1 = op.tile([128, NS], F32, tag="o1")
                nc.scalar.activation(
                    o1[:], acc[mb][:], mybir.ActivationFunctionType.Copy, scale=s_ap
                )
                o2 = op.tile([128, NS], F32, tag="o2")
                nc.vector.tensor_tensor(o2[:], o1[:], bias_b[:], op=mybir.AluOpType.add)
                nc.sync.dma_start(out_t[mb * 128 : (mb + 1) * 128, :], o2[:])

    nc.compile()
    return nc


_NC = None


def _get_nc():
    global _NC
    if _NC is None:
        _NC = _build_program()
    return _NC


def _quantize(a):
    """Exactly the reference's quantization: scale = amax/127 (f32 IEEE),
    q = clip(round-half-even(a / scale), -127, 127)."""
    amax = np.float32(np.max(np.abs(a)))
    scale = amax / np.float32(127.0)
    q = np.clip(np.round((a / scale).astype(np.float32)), -127.0, 127.0)
    return q.astype(ml_dtypes.bfloat16), scale


def kernel(x, weight, bias, _trace=False):
    x = np.asarray(x, dtype=np.float32)
    weight = np.asarray(weight, dtype=np.float32)
    bias = np.asarray(bias, dtype=np.float32)

    qx, sx = _quantize(x)
    qw, sw = _quantize(weight)
    s = sx * sw
    scl = np.array([[s, sx, sw, 0.0]], dtype=np.float32)

    qxt = np.ascontiguousarray(qx.T)  # [K, M]
    qwt = np.ascontiguousarray(qw.T)  # [K, N]

    in_maps = []
    for c in range(8):
        i, j = divmod(c, PN)
        in_maps.append(
            {
                "qxt_sh": np.ascontiguousarray(qxt[:, i * MS : (i + 1) * MS]),
                "qwt_sh": np.ascontiguousarray(qwt[:, j * NS : (j + 1) * NS]),
                "b_sh": bias[j * NS : (j + 1) * NS].reshape(1, NS),
                "scl": scl,
            }
        )

    nc = _get_nc()
    res = run_bass_kernel_spmd(nc, in_maps, core_ids=list(range(8)), trace=_trace)

    out = np.empty((M, N), np.float32)
    for c in range(8):
        i, j = divmod(c, PN)
        out[i * MS : (i + 1) * MS, j * NS : (j + 1) * NS] = res.results[c]["out_sh"]
    if _trace:
        return out, res
    return out


# revision 4
# speedup vs baseline: 1.0175x; 1.0175x over previous
"""Trainium2 kernel for nn_LinearMem: bit-sliced int8-quantized linear layer.

Math: the reference splits round(x/sx) and round(w.T/sw) into two's-complement
bit-planes (widths 1,1,2,4) and recombines 16 per-slice-pair matmuls with
2^shift weights.  That recombination is exactly sum_i 2^sh_i * plane_i == q,
so the whole einsum equals qx @ qw^T with qx = round(x/sx), qw = round(w/sw)
(clip to +-127 is a no-op since |x|/sx <= 127 by construction).  Every product
and partial sum is an integer < 2^24, so a bf16 x bf16 matmul with f32 PSUM
accumulation reproduces the reference bitwise (int8 values are exact in bf16).

Quantization itself needs an exact IEEE f32 divide to match the reference's
rounding; Trainium has no divide instruction on any engine (DVE/ACT/GPSIMD ISA
all reject AluOpType.divide), so the int8 quantization + shard layout prep is
done host-side (as in real quantized inference, where weights are quantized
offline).  The device does all 17 GFLOP of matmul plus dequantize + bias.

Distribution (8 NeuronCores, tensor-parallel 2x4 grid):
  core c = (i, j): i = c//4 selects token rows (M/2 = 1024), j = c%4 selects
  out_features (N/4 = 512).  Each core receives its pre-transposed [K, M_c]
  activation slice and [K, N_c] weight slice in bf16, accumulates
  out[m, n] = sum_k qxT[k, m] * qwT[k, n] over 16 K-blocks into 8 persistent
  PSUM banks, then dequantizes (ACT scale) + adds bias (DVE) and writes its
  [1024, 512] f32 output block.  Host reassembles the 2x4 grid.
"""

import sys

if "/opt/trn_rl_repo" not in sys.path:
    sys.path.insert(0, "/opt/trn_rl_repo")

import ml_dtypes
import numpy as np

import concourse.bacc as bacc
import concourse.mybir as mybir
import concourse.tile as tile
from concourse.bass_utils import run_bass_kernel_spmd

M, K, N = 2048, 2048, 2048
PM, PN = 2, 4  # grid: M split PM ways, N split PN ways
MS, NS = M // PM, N // PN  # per-core shard sizes: 1024, 512

F32 = mybir.dt.float32
BF16 = mybir.dt.bfloat16


def _build_program():
    nc = bacc.Bacc("TRN2", target_bir_lowering=False, debug=False, num_devices=8)

    qx_in = nc.dram_tensor("qxt_sh", [K, MS], BF16, kind="ExternalInput")
    qw_in = nc.dram_tensor("qwt_sh", [K, NS], BF16, kind="ExternalInput")
    b_in = nc.dram_tensor("b_sh", [1, NS], F32, kind="ExternalInput")
    scl_in = nc.dram_tensor("scl", [1, 4], F32, kind="ExternalInput")
    out_t = nc.dram_tensor("out_sh", [MS, NS], F32, kind="ExternalOutput")

    MT = MS // 128  # 8 m-tiles
    KT = K // 128  # 16 k-blocks

    with tile.TileContext(nc) as tc:
        with (
            tc.tile_pool(name="const", bufs=1) as const,
            tc.tile_pool(name="wpool", bufs=1) as wpool,
            tc.tile_pool(name="xpool", bufs=3) as xpool,
            tc.tile_pool(name="out", bufs=3) as op,
            tc.tile_pool(name="psum", bufs=3, space="PSUM") as ps,
        ):
            # weight tiles: preload all 16 [128, NS] K-blocks (contiguous DMAs)
            # on the ACT HWDGE ring; x tiles stream on the SP ring.
            wt = []
            for kb in range(KT):
                w = wpool.tile([128, NS], BF16, tag=f"wt{kb}", name=f"wt{kb}")
                nc.scalar.dma_start(w[:], qw_in[kb * 128 : (kb + 1) * 128, :])
                wt.append(w)

            # constants via SWDGE (gpsimd) to keep the HWDGE rings free
            scl_row = const.tile([1, 4], F32, tag="scl_row")
            nc.gpsimd.dma_start(scl_row[:], scl_in[:])
            sclb = const.tile([128, 4], F32, tag="sclb")
            nc.gpsimd.partition_broadcast(sclb[:], scl_row[:], channels=128)
            s_ap = sclb[:, 0:1]  # dequant scale sx*sw

            bias_row = const.tile([1, NS], F32, tag="bias_row")
            nc.gpsimd.dma_start(bias_row[:], b_in[:])
            bias_b = const.tile([128, NS], F32, tag="bias_b")
            nc.gpsimd.partition_broadcast(bias_b[:], bias_row[:], channels=128)

            qx_v = qx_in.rearrange("(t p) m -> t p m", p=128)  # [KT, 128, MS]
            for mb in range(MT):
                # [128 k-part, KT, 128 m] slice of this mb's columns
                xmb = xpool.tile([128, KT, 128], BF16, tag="xmb")
                nc.sync.dma_start(
                    xmb[:], qx_v[:, :, mb * 128 : (mb + 1) * 128].rearrange("t p m -> p t m")
                )
                acc = ps.tile([128, NS], F32, tag="acc")
                for kb in range(KT):
                    nc.tensor.matmul(
                        acc[:],
                        xmb[:, kb, :],
                        wt[kb][:],
                        start=(kb == 0),
                        stop=(kb == KT - 1),
                    )
                o1 = op.tile([128, NS], F32, tag="o1")
                nc.scalar.activation(
                    o1[:], acc[:], mybir.ActivationFunctionType.Copy, scale=s_ap
                )
                o2 = op.tile([128, NS], F32, tag="o2")
                nc.vector.tensor_tensor(o2[:], o1[:], bias_b[:], op=mybir.AluOpType.add)
                nc.gpsimd.dma_start(out_t[mb * 128 : (mb + 1) * 128, :], o2[:])

    nc.compile()
    return nc


_NC = None


def _get_nc():
    global _NC
    if _NC is None:
        _NC = _build_program()
    return _NC


def _quantize(a):
    """Exactly the reference's quantization: scale = amax/127 (f32 IEEE),
    q = clip(round-half-even(a / scale), -127, 127)."""
    amax = np.float32(np.max(np.abs(a)))
    scale = amax / np.float32(127.0)
    q = np.clip(np.round((a / scale).astype(np.float32)), -127.0, 127.0)
    return q.astype(ml_dtypes.bfloat16), scale


def kernel(x, weight, bias, _trace=False):
    x = np.asarray(x, dtype=np.float32)
    weight = np.asarray(weight, dtype=np.float32)
    bias = np.asarray(bias, dtype=np.float32)

    qx, sx = _quantize(x)
    qw, sw = _quantize(weight)
    s = sx * sw
    scl = np.array([[s, sx, sw, 0.0]], dtype=np.float32)

    qxt = np.ascontiguousarray(qx.T)  # [K, M]
    qwt = np.ascontiguousarray(qw.T)  # [K, N]

    in_maps = []
    for c in range(8):
        i, j = divmod(c, PN)
        in_maps.append(
            {
                "qxt_sh": np.ascontiguousarray(qxt[:, i * MS : (i + 1) * MS]),
                "qwt_sh": np.ascontiguousarray(qwt[:, j * NS : (j + 1) * NS]),
                "b_sh": bias[j * NS : (j + 1) * NS].reshape(1, NS),
                "scl": scl,
            }
        )

    nc = _get_nc()
    res = run_bass_kernel_spmd(nc, in_maps, core_ids=list(range(8)), trace=_trace)

    out = np.empty((M, N), np.float32)
    for c in range(8):
        i, j = divmod(c, PN)
        out[i * MS : (i + 1) * MS, j * NS : (j + 1) * NS] = res.results[c]["out_sh"]
    if _trace:
        return out, res
    return out
